# revision 14
# baseline (speedup 1.0000x reference)
"""GCN layer (gather + segment-mean + linear) as a Bass/Tile kernel on 8 TRN2 cores.

Strategy (edge parallel, sharded by destination node):
  - Each core owns a contiguous range of N/8 destination nodes and processes
    exactly the edges landing in that range; no cross-core reduction needed.
  - h and r are carried as single bf16 (rel-err budget is 2e-2; bf16 gives
    ~1e-3).  h rows are 256B, gathered per edge with GPSIMD dma_gather in
    large batches (thousands of indices per call) to amortize the SWDGE
    fixed overhead.  dma_gather indices are int16, so the chunk stream is
    segregated into "lo" (src < 32768) and "hi" runs per node group.
  - Segment-sum over a 128-edge chunk is one bf16 PE matmul: S.T @ [hg | r]
    where S[e, n] = (dst_local[e] == n).  S is built on DVE in batches of
    K_EQ chunks with a single tensor_tensor is_equal against a constant
    iota tile (host-precomputed dst values broadcast along the node axis).
  - The gathered h plane and the streamed r plane live in one SBUF tile
    [128, 2, CH, 128] so each chunk needs only one ldweights + one 256-col
    matmul (rhs AP [128, 2, 128] spans both planes).
  - PSUM accumulates [node, 256] f32 per 128-node group; epilogue sums the
    two halves, transposes on PE, GEMMs with W^T (f32), applies
    1/max(indeg,1) and bias in one scalar_tensor_tensor, DMAs out.
  - Groups are processed in supergroups of SG_N groups; within a supergroup
    the chunk stream is [all lo runs][all hi runs] so the gather needs just
    two large calls per supergroup.  PSUM accumulation interleaves across
    the supergroup's groups.
  - One SPMD program shared by all cores: per-(group, half) chunk counts are
    padded up to the max over the 8 cores; pad edges carry dst=-1 so their
    one-hot row is zero and they contribute nothing.
"""

import numpy as np

N_NODES = 50000
D = 128
N_CORES = 8
HALF = 32768  # int16 index reach for dma_gather
K_EQ = 16  # chunks per batched is_equal S-build
SG_N = 3  # groups per supergroup
GCALL = 8  # max chunks (x128 idxs) per dma_gather call (ring capacity bound)


def _preprocess(src, dst, h, r, W, b, n_cores=N_CORES, n_nodes=N_NODES, half=HALF):
    import ml_dtypes

    bf16 = ml_dtypes.bfloat16
    src = np.asarray(src).astype(np.int64)
    dst = np.asarray(dst).astype(np.int64)
    h = np.ascontiguousarray(np.asarray(h, dtype=np.float32))
    r = np.asarray(r, dtype=np.float32)
    W = np.asarray(W, dtype=np.float32)
    b = np.asarray(b, dtype=np.float32)
    E = src.shape[0]
    npc = n_nodes // n_cores
    G = -(-npc // 128)

    counts = np.bincount(dst, minlength=n_nodes).astype(np.float32)
    inv = (1.0 / np.maximum(counts, 1.0)).astype(np.float32)

    h_bf = h.astype(bf16)  # [N, 128]
    r_bf = r.astype(bf16)

    core = dst // npc
    nl = dst % npc
    g = nl // 128
    is_hi = (src >= half).astype(np.int64)
    key = (core * G + g) * 2 + is_hi
    nkeys = n_cores * G * 2

    cnt = np.bincount(key, minlength=nkeys)
    chunks = -(-cnt // 128)
    caps = chunks.reshape(n_cores, G, 2).max(axis=0)  # [G, 2] chunks per run
    caps = np.maximum(caps, 1)

    # Supergroups of SG_N groups; chunk stream per supergroup:
    # [lo runs of each group in order][hi runs of each group in order].
    sgs = [list(range(s, min(s + SG_N, G))) for s in range(0, G, SG_N)]
    run_chunk_base = np.zeros((G, 2), np.int64)
    pos_c = 0
    sg_info = []  # (base, nlo, nhi, groups)
    for sg in sgs:
        base = pos_c
        nlo = 0
        for gg in sg:
            run_chunk_base[gg, 0] = pos_c
            pos_c += int(caps[gg, 0])
            nlo += int(caps[gg, 0])
        nhi = 0
        for gg in sg:
            run_chunk_base[gg, 1] = pos_c
            pos_c += int(caps[gg, 1])
            nhi += int(caps[gg, 1])
        sg_info.append((base, nlo, nhi, list(sg)))
    total_chunks = pos_c
    P_edges = total_chunks * 128

    order = np.argsort(key, kind="stable")
    key_s = key[order]
    run_starts = np.zeros(nkeys, np.int64)
    run_starts[1:] = np.cumsum(cnt)[:-1]
    offs = np.arange(E, dtype=np.int64) - run_starts[key_s]
    ecore = key_s // (G * 2)
    eg = (key_s // 2) % G
    ehalf = key_s % 2
    pos = run_chunk_base[eg, ehalf] * 128 + offs  # position in the padded stream

    per_core = []
    for c in range(n_cores):
        m = ecore == c
        p = pos[m]
        e = order[m]
        # scatter r rows: partition = slot in chunk, cols = chunk*128 + feat
        rT2 = np.zeros((128, total_chunks, 128), bf16)
        rT2[p % 128, p // 128, :] = r_bf[e]
        idx16 = np.zeros(P_edges, np.int16)
        idx16[p] = (src[e] - half * ehalf[m]).astype(np.int16)
        dff = np.full((128, total_chunks), -1.0, np.float32)
        dff[p % 128, p // 128] = (nl[e] - eg[m] * 128).astype(np.float32)
        idxw = np.tile(np.ascontiguousarray(idx16.reshape(-1, 16).T), (8, 1))
        invp = np.zeros(G * 128, np.float32)
        invp[:npc] = inv[c * npc : (c + 1) * npc]
        inv_t = np.ascontiguousarray(invp.reshape(G, 128).T)
        per_core.append(
            {
                "h": h_bf,
                "rT": np.ascontiguousarray(rT2.reshape(128, total_chunks * 128)),
                "idxw": np.ascontiguousarray(idxw),
                "dff": dff.astype(bf16),
                "invt": inv_t,
                "iota": np.tile(
                    np.repeat(np.arange(128, dtype=np.float32), K_EQ), (128, 1)
                ).astype(bf16),
                "wt": np.ascontiguousarray(W.T),
                "bb": np.tile(b, (128, 1)),
                "ident": np.eye(128, dtype=np.float32),
            }
        )
    return per_core, caps, sg_info, total_chunks, npc, G


def _build(caps, sg_info, total_chunks, npc, G, n_nodes=N_NODES, half=HALF, nq=4):
    from contextlib import ExitStack

    import concourse.bacc as bacc
    import concourse.mybir as mybir
    import concourse.tile as tile

    f32 = mybir.dt.float32
    bf16 = mybir.dt.bfloat16
    i16 = mybir.dt.int16
    ADD = mybir.AluOpType.add
    MUL = mybir.AluOpType.mult
    ISEQ = mybir.AluOpType.is_equal

    CHmax = max(nlo + nhi for (_, nlo, nhi, _) in sg_info)

    nc = bacc.Bacc(
        "TRN2", target_bir_lowering=False, debug=False, num_swdge_queues=nq
    )
    h_d = nc.dram_tensor("h", [n_nodes, 128], bf16, kind="ExternalInput")
    r_d = nc.dram_tensor("rT", [128, total_chunks * 128], bf16, kind="ExternalInput")
    idx_d = nc.dram_tensor("idxw", [128, total_chunks * 8], i16, kind="ExternalInput")
    dff_d = nc.dram_tensor("dff", [128, total_chunks], bf16, kind="ExternalInput")
    inv_d = nc.dram_tensor("invt", [128, G], f32, kind="ExternalInput")
    iota_d = nc.dram_tensor("iota", [128, K_EQ * 128], bf16, kind="ExternalInput")
    wt_d = nc.dram_tensor("wt", [128, 128], f32, kind="ExternalInput")
    bb_d = nc.dram_tensor("bb", [128, 128], f32, kind="ExternalInput")
    id_d = nc.dram_tensor("ident", [128, 128], f32, kind="ExternalInput")
    out_d = nc.dram_tensor("out", [npc, 128], f32, kind="ExternalOutput")

    h_lo_v = h_d[0:half, :]
    h_hi_v = h_d[half:n_nodes, :]

    # chunk -> group map and first/last flags
    gof = [0] * total_chunks
    first = [False] * total_chunks
    last = [False] * total_chunks
    ng_chunks = [int(caps[g, 0] + caps[g, 1]) for g in range(G)]
    seen = [0] * G
    for base, nlo, nhi, groups in sg_info:
        c = base
        for hf in (0, 1):
            for gg in groups:
                for _ in range(int(caps[gg, hf])):
                    gof[c] = gg
                    if seen[gg] == 0:
                        first[c] = True
                    seen[gg] += 1
                    if seen[gg] == ng_chunks[gg]:
                        last[c] = True
                    c += 1

    with tile.TileContext(nc) as tc, ExitStack() as ctx:
        const = ctx.enter_context(tc.tile_pool(name="const", bufs=1))
        hrp = ctx.enter_context(tc.tile_pool(name="hr", bufs=2))
        sp = ctx.enter_context(tc.tile_pool(name="s", bufs=6))
        ftp = ctx.enter_context(tc.tile_pool(name="ft", bufs=4))
        outp = ctx.enter_context(tc.tile_pool(name="o", bufs=3))
        psA = ctx.enter_context(tc.tile_pool(name="psA", bufs=5, space="PSUM"))
        psT = ctx.enter_context(tc.tile_pool(name="psT", bufs=1, space="PSUM"))
        psO = ctx.enter_context(tc.tile_pool(name="psO", bufs=2, space="PSUM"))

        iota_t = const.tile([128, 128, K_EQ], bf16)
        nc.sync.dma_start(
            iota_t[:], iota_d[:].rearrange("p (n j) -> p n j", j=K_EQ)
        )
        wt_t = const.tile([128, 128], f32)
        nc.sync.dma_start(wt_t[:], wt_d[:])
        bb_t = const.tile([128, 128], f32)
        nc.sync.dma_start(bb_t[:], bb_d[:])
        id_t = const.tile([128, 128], f32)
        nc.sync.dma_start(id_t[:], id_d[:])
        inv_t = const.tile([128, G], f32)
        nc.sync.dma_start(inv_t[:], inv_d[:])
        dff_t = const.tile([128, total_chunks], bf16)
        nc.sync.dma_start(dff_t[:], dff_d[:])
        idx_t = const.tile([128, total_chunks * 8], i16)
        nc.sync.dma_start(idx_t[:], idx_d[:])

        qctr = 0
        accs = {}
        Sb = None
        for base, nlo, nhi, groups in sg_info:
            CH = nlo + nhi
            HR = hrp.tile([128, 2, CHmax, 128], bf16, tag="hr")
            for seg0, seg1, hsrc in ((0, nlo, h_lo_v), (nlo, CH, h_hi_v)):
                s = seg0
                while s < seg1:
                    n = min(GCALL, seg1 - s)
                    nc.gpsimd.dma_gather(
                        HR[:, 0, s : s + n, :],
                        hsrc,
                        idx_t[:, (base + s) * 8 : (base + s + n) * 8],
                        n * 128,
                        n * 128,
                        128,
                        queue_num=qctr % nq,
                    )
                    qctr += 1
                    s += n
            nc.sync.dma_start(
                HR[:, 1, 0:CH, :].rearrange("p c f -> p (c f)"),
                r_d[:, base * 128 : (base + CH) * 128],
            )
            for lc in range(CH):
                c = base + lc
                jj = lc % K_EQ
                if jj == 0:
                    kk = min(K_EQ, CH - lc)
                    Sb = sp.tile([128, 128, K_EQ], bf16, tag="sb")
                    dfb = (
                        dff_t[:, c : c + kk]
                        .broadcast_to([128, kk, 128])
                        .rearrange("p a b -> p b a")
                    )
                    nc.vector.tensor_tensor(
                        Sb[:, :, 0:kk], iota_t[:, :, 0:kk], dfb, ISEQ
                    )
                gg = gof[c]
                if first[c]:
                    accs[gg] = psA.tile([128, 256], f32, tag="acc", name=f"acc{gg}")
                nc.tensor.matmul(
                    accs[gg][:],
                    Sb[:, :, jj],
                    HR[:, :, lc, :],
                    start=first[c],
                    stop=last[c],
                )
                if last[c]:
                    acc = accs.pop(gg)
                    ft = ftp.tile([128, 128], f32, tag="ft")
                    nc.scalar.copy(ft[:], acc[:, 0:128])
                    ftb = ftp.tile([128, 128], f32, tag="ftb")
                    nc.vector.tensor_tensor(ftb[:], ft[:], acc[:, 128:256], ADD)
                    pt = psT.tile([128, 128], f32)
                    nc.tensor.transpose(pt[:], ftb[:], id_t[:])
                    ptc = ftp.tile([128, 128], f32, tag="ptc")
                    nc.scalar.copy(ptc[:], pt[:])
                    po = psO.tile([128, 128], f32)
                    nc.tensor.matmul(po[:], ptc[:], wt_t[:], start=True, stop=True)
                    ot = outp.tile([128, 128], f32)
                    nc.vector.scalar_tensor_tensor(
                        ot[:], po[:], inv_t[:, gg : gg + 1], bb_t[:], MUL, ADD
                    )
                    rows = min(128, npc - gg * 128)
                    nc.sync.dma_start(
                        out_d[gg * 128 : gg * 128 + rows, :], ot[0:rows, :]
                    )

    nc.compile()
    return nc


LAST_RESULT = None


def kernel(src, dst, h, r, W, b, _trace=False, _tmpdir=None):
    global LAST_RESULT
    from concourse.bass_utils import run_bass_kernel_spmd

    per_core, caps, sg_info, total_chunks, npc, G = _preprocess(src, dst, h, r, W, b)
    nc = _build(caps, sg_info, total_chunks, npc, G)
    kwargs = {}
    if _trace:
        kwargs = dict(trace=True, tmpdir=_tmpdir)
    res = run_bass_kernel_spmd(nc, per_core, list(range(N_CORES)), **kwargs)
    LAST_RESULT = res
    out = np.concatenate([res.results[c]["out"] for c in range(N_CORES)], axis=0)
    return out.astype(np.float32)
